# revision 1
# baseline (speedup 1.0000x reference)
"""DiceLoss (CondInst-style dynamic mask head) Trainium2 kernel.

Data-parallel over batch B=8: one image per NeuronCore. Per image:
  - gather per-object 1x1-conv weights from conv_weight at ind (host, tiny)
  - conv1: [10 -> 8] per object (relu), conv2: [8 -> 8] (relu),
    conv3: [8 -> 1] (sigmoid) over all HW=16384 pixels, K=32 objects
  - dice partial sums sum(p*t), sum(p*p) per image (sum(t*t) is
    pred-independent and computed on host)

Host folds the relative-coordinate channels into conv1's bias (they are
affine in the pixel grid), builds block-diagonal weights so all 32 objects
run as dense 128-contraction matmuls, pre-masks target, and forces
sigmoid->0 for masked objects via a large negative conv3 bias.

Device layout (per core), processed in 8 groups of 4 hw-chunks (512 px):
  conv1: two 16-object halves row-packed via tile_position (0,0)/(32,0),
         so both halves stream through the PE concurrently; 2-bank-wide
         PSUM tiles halve the evacuation op count.
  conv2: weight-batched (all 4 chunks of half A, then half B).
  conv3: col-tiled - chunk j lands at PSUM partitions 32j, so 4 chunks
         pack one bank and sigmoid runs on full 128-partition tiles.
  dice:  pt/pp products on DVE, summed over hw by PE ones-matmuls into a
         single shared PSUM bank (cols 0:256 pt, 256:512 pp).
"""

import numpy as np
import ml_dtypes

import concourse.bass as bass
import concourse.mybir as mybir
import concourse.tile as tile
from concourse.bass_utils import run_bass_kernel_spmd

BF16 = mybir.dt.bfloat16
F32 = mybir.dt.float32

B, C, K, H, W = 8, 8, 32, 128, 128
HW = H * W
CW = 169
CHUNK = 512
NGROUP = 8           # groups of 4 chunks
N_CORES = 8

_NEG_BIG = 30000.0   # sigmoid(z - 30000) == 0 for any realistic z


# ---------------------------------------------------------------------------
# Workarounds for this walrus build's 1-sem-wait-per-instruction encoding
# limit: split Tile's multi-wait drain and spill excess waits onto NoOps.
# ---------------------------------------------------------------------------
def _drain_and_barrier_split(self, tick_clock, wait_clock):
    from concourse.tile import ScopedClock

    nc = self.nc
    drain_inst = nc.sync.drain()
    wait_clock.add_sem_waits(
        drain_inst.ins, ScopedClock({None: tick_clock.global_clock})
    )
    si = drain_inst.ins.sync_info
    waits = list(si.on_wait) if si is not None else []
    if len(waits) > 1:
        drain_inst.ins.sync_info = None
        handles = list(self.sems.allocated().values())
        by_num = {h.num: h for h in handles}
        by_name = {h.name: h for h in handles}
        for w_ in waits:
            h = by_num.get(w_.id) or by_name.get(w_.ant_name)
            assert h is not None, f"no semaphore handle for {w_}"
            assert w_.wait_mode == "sem-ge-imm", w_.wait_mode
            nc.sync.wait_ge(h, w_.wait_value)
    nc.all_engine_barrier()
    popped = nc._tile_sem_poison_stack.pop()
    assert popped is self._sem_poison
    nc.clear_and_free_semaphores(list(self.sems.allocated().values()))
    nc.all_engine_barrier()


tile.TileContext._drain_and_barrier = _drain_and_barrier_split


def split_excess_waits(nc, register=True):
    for f in nc.m.functions:
        for bb in f.blocks:
            out = []
            changed = False
            for inst in bb.instructions:
                si = inst.sync_info
                waits = list(si.on_wait) if si is not None else []
                if len(waits) > 1:
                    keep, spill = waits[:1], waits[1:]
                    for i, w_ in enumerate(spill):
                        nop = mybir.InstNoOp(
                            name=f"{inst.name}_wspill{i}",
                            engine=inst.engine,
                            sync_info=mybir.SyncInfo(on_wait=[w_], on_update=[]),
                            bass_nofuse=True,
                        )
                        if register:
                            nc.register_instruction(nop, overwrite=True)
                        out.append(nop)
                    inst.sync_info = mybir.SyncInfo(
                        on_wait=keep, on_update=list(si.on_update)
                    )
                    changed = True
                out.append(inst)
            if changed:
                bb.instructions = out


# ---------------------------------------------------------------------------
# Device kernel
# ---------------------------------------------------------------------------
def build_nc():
    nc = bass.Bass()
    f10_d = nc.declare_dram_parameter("f10", [10, HW], BF16, False)
    w1t_d = nc.declare_dram_parameter("w1t", [42, 128], BF16, False)
    w2t_d = nc.declare_dram_parameter("w2t", [128, 256], BF16, False)
    w3t_d = nc.declare_dram_parameter("w3t", [128, 64], BF16, False)
    b12_d = nc.declare_dram_parameter("b12", [128, 4], F32, False)
    b3_d = nc.declare_dram_parameter("b3", [128, 1], F32, False)
    tpk_d = nc.declare_dram_parameter("tpk", [128, 4096], BF16, False)
    red_d = nc.declare_dram_parameter("red", [1, 512], F32, True)

    RELU = mybir.ActivationFunctionType.Relu
    SIGM = mybir.ActivationFunctionType.Sigmoid
    ADD = mybir.AluOpType.add
    MAX = mybir.AluOpType.max

    with tile.TileContext(nc) as tc:
        with (
            tc.tile_pool(name="const", bufs=1) as const,
            tc.tile_pool(name="h1p", bufs=2) as h1p,
            tc.tile_pool(name="h2p", bufs=5) as h2p,
            tc.tile_pool(name="predp", bufs=2) as predp,
            tc.tile_pool(name="prodp", bufs=2) as prodp,
            tc.tile_pool(name="ps1ap", bufs=1, space="PSUM") as ps1ap,
            tc.tile_pool(name="ps1bp", bufs=1, space="PSUM") as ps1bp,
            tc.tile_pool(name="ps2p", bufs=1, space="PSUM") as ps2p,
            tc.tile_pool(name="ps3p", bufs=1, space="PSUM") as ps3p,
            tc.tile_pool(name="psredp", bufs=1, space="PSUM") as psredp,
        ):
            w1_sb = const.tile([42, 128], BF16)
            nc.gpsimd.dma_start(out=w1_sb[:], in_=w1t_d[:])
            w2_sb = const.tile([128, 256], BF16)
            nc.gpsimd.dma_start(out=w2_sb[:], in_=w2t_d[:])
            w3_sb = const.tile([128, 64], BF16)
            nc.gpsimd.dma_start(out=w3_sb[:], in_=w3t_d[:])
            b12_sb = const.tile([128, 4], F32)
            nc.gpsimd.dma_start(out=b12_sb[:], in_=b12_d[:])
            b3_sb = const.tile([128, 1], F32)
            nc.gpsimd.dma_start(out=b3_sb[:], in_=b3_d[:])
            f_sb = const.tile([42, HW], BF16)
            nc.gpsimd.dma_start(out=f_sb[0:10, :], in_=f10_d[:])
            nc.sync.dma_start(out=f_sb[32:42, :], in_=f10_d[:])
            tpk_sb = const.tile([128, 4096], BF16)
            nc.gpsimd.dma_start(out=tpk_sb[:], in_=tpk_d[:])

            ones_sb = const.tile([128, 1], BF16)
            nc.vector.memset(ones_sb, 1.0)
            # shared accumulator bank: cols 0:256 pt, 256:512 pp
            red = psredp.tile([1, 512], F32)

            def evac_relu(dst, src, bias_ap, on_act):
                if on_act:
                    nc.scalar.activation(
                        out=dst, in_=src, func=RELU, bias=bias_ap
                    )
                else:
                    nc.vector.tensor_scalar(
                        out=dst, in0=src, scalar1=bias_ap, scalar2=0.0,
                        op0=ADD, op1=MAX,
                    )

            first_red = [True]

            def reduce_into(cols, prod):
                # red[0, cols] += column sums of prod (two N=256 matmuls)
                for h_ in range(2):
                    nc.tensor.matmul(
                        red[0:1, cols],
                        ones_sb[:],
                        prod[:, bass.ts(h_, 256)],
                        start=first_red[0],
                        stop=False,
                        skip_group_check=True,
                    )
                    first_red[0] = False

            for g in range(NGROUP):
                c0 = 4 * g
                # ---- conv1: row-packed halves, 2 chunk-pairs -> wide psum
                ps1a = [None, None]
                ps1b = [None, None]
                h1a = [None, None]
                h1b = [None, None]
                for p in range(2):
                    ps1a[p] = ps1ap.tile([128, 1024], F32, tag="ps1a", name="ps1a")
                    ps1b[p] = ps1bp.tile([128, 1024], F32, tag="ps1b", name="ps1b")
                    for i in range(2):
                        cs = bass.ts(c0 + 2 * p + i, CHUNK)
                        ncol = bass.ts(i, CHUNK)
                        nc.tensor.matmul(
                            ps1a[p][:, ncol], w1_sb[0:10, :], f_sb[0:10, cs],
                            start=True, stop=True, tile_position=(0, 0),
                        )
                        nc.tensor.matmul(
                            ps1b[p][:, ncol], w1_sb[32:42, :], f_sb[32:42, cs],
                            start=True, stop=True, tile_position=(32, 0),
                        )
                    h1a[p] = h1p.tile([128, 1024], BF16, tag="h1a", name="h1a")
                    evac_relu(h1a[p][:], ps1a[p][:], b12_sb[:, 0:1],
                              on_act=(p == 1))
                    h1b[p] = h1p.tile([128, 1024], BF16, tag="h1b", name="h1b")
                    evac_relu(h1b[p][:], ps1b[p][:], b12_sb[:, 1:2],
                              on_act=(p == 0))

                # ---- conv2: weight-batched (A over 4 chunks, then B)
                h2a = [None] * 4
                h2b = [None] * 4
                for p in range(2):
                    ps2 = ps2p.tile([128, 1024], F32, tag="ps2")
                    for i in range(2):
                        j = 2 * p + i
                        nc.tensor.matmul(
                            ps2[:, bass.ts(i, CHUNK)], w2_sb[:, 0:128],
                            h1a[p][:, bass.ts(i, CHUNK)],
                            start=True, stop=True,
                        )
                    for i in range(2):
                        j = 2 * p + i
                        h2a[j] = h2p.tile([128, CHUNK], BF16, tag="h2a", name="h2a")
                        evac_relu(h2a[j][:], ps2[:, bass.ts(i, CHUNK)],
                                  b12_sb[:, 2:3], on_act=(j % 2 == 0))
                for p in range(2):
                    ps2 = ps2p.tile([128, 1024], F32, tag="ps2")
                    for i in range(2):
                        j = 2 * p + i
                        nc.tensor.matmul(
                            ps2[:, bass.ts(i, CHUNK)], w2_sb[:, 128:256],
                            h1b[p][:, bass.ts(i, CHUNK)],
                            start=True, stop=True,
                        )
                    for i in range(2):
                        j = 2 * p + i
                        h2b[j] = h2p.tile([128, CHUNK], BF16, tag="h2b", name="h2b")
                        evac_relu(h2b[j][:], ps2[:, bass.ts(i, CHUNK)],
                                  b12_sb[:, 3:4], on_act=(j % 2 == 1))

                # ---- conv3: batched A then B, col-tiled into one bank
                ps3 = ps3p.tile([128, CHUNK], F32, tag="ps3")
                for j in range(4):
                    nc.tensor.matmul(
                        ps3[32 * j : 32 * j + 32, :], w3_sb[:, 0:32],
                        h2a[j][:],
                        start=True, stop=False,
                        tile_position=(0, 32 * j),
                        skip_group_check=True,
                    )
                for j in range(4):
                    nc.tensor.matmul(
                        ps3[32 * j : 32 * j + 32, :], w3_sb[:, 32:64],
                        h2b[j][:],
                        start=False, stop=True,
                        tile_position=(0, 32 * j),
                        skip_group_check=True,
                    )

                # ---- sigmoid + dice products
                pred = predp.tile([128, CHUNK], BF16, tag="pred")
                nc.scalar.activation(
                    out=pred[:], in_=ps3[:], func=SIGM, bias=b3_sb[:, 0:1]
                )
                tgt = tpk_sb[:, bass.ts(g, CHUNK)]
                pt_s = prodp.tile([128, CHUNK], BF16, tag="pt_s")
                nc.vector.tensor_mul(out=pt_s[:], in0=pred[:], in1=tgt)
                reduce_into(slice(0, 256), pt_s)
                pp_s = prodp.tile([128, CHUNK], BF16, tag="pp_s")
                nc.vector.tensor_mul(out=pp_s[:], in0=pred[:], in1=pred[:])
                reduce_into(slice(256, 512), pp_s)

            red_sb = const.tile([1, 512], F32)
            nc.scalar.copy(out=red_sb[:], in_=red[:])
            nc.gpsimd.dma_start(out=red_d[:], in_=red_sb[:])
    split_excess_waits(nc)
    return nc


# ---------------------------------------------------------------------------
# Host-side input preparation (numpy, per image)
# ---------------------------------------------------------------------------
def prep_inputs(seg_feat, conv_weight, mask, ind, target):
    seg_feat = np.asarray(seg_feat)
    conv_weight = np.asarray(conv_weight)
    mask = np.asarray(mask)
    ind = np.asarray(ind).astype(np.int64)
    target = np.asarray(target)

    cw = conv_weight.reshape(B, CW, HW)
    w = np.take_along_axis(cw, ind[:, None, :], axis=2)  # [B, CW, K]
    w = np.ascontiguousarray(w.transpose(0, 2, 1)).astype(np.float32)  # [B,K,CW]

    c1w = w[..., 0:80].reshape(B, K, C, C + 2)
    c1b = w[..., 80:88]
    c2w = w[..., 88:152].reshape(B, K, C, C)
    c2b = w[..., 152:160]
    c3w = w[..., 160:168].reshape(B, K, C)
    c3b = w[..., 168]

    x = (ind % W).astype(np.float32) / W
    y = (ind // W).astype(np.float32) / H
    b1eff = c1b - c1w[..., 8] * x[:, :, None] - c1w[..., 9] * y[:, :, None]

    mf = mask.astype(np.float32)
    b3eff = c3b - _NEG_BIG * (1.0 - mf)

    xg = (np.arange(HW, dtype=np.float32) % W) / W
    yg = (np.arange(HW, dtype=np.float32) // W) / H

    bf = ml_dtypes.bfloat16
    in_maps = []
    tt_host = np.empty(B, np.float64)
    for b in range(B):
        f10 = np.concatenate(
            [seg_feat[b].reshape(C, HW), xg[None], yg[None]], axis=0
        ).astype(bf)

        w1half = c1w[b].transpose(2, 0, 1).reshape(C + 2, K * C)  # [10, 256]
        w1t = np.zeros((42, 128), np.float32)
        w1t[0:10, :] = w1half[:, 0:128]
        w1t[32:42, :] = w1half[:, 128:256]
        w1t = w1t.astype(bf)

        w2t = np.zeros((128, 256), np.float32)
        for half in range(2):
            for kl in range(16):
                blk = c2w[b, half * 16 + kl].T  # [c, o]
                w2t[kl * 8 : kl * 8 + 8,
                    half * 128 + kl * 8 : half * 128 + kl * 8 + 8] = blk
        w2t = w2t.astype(bf)

        w3t = np.zeros((128, 64), np.float32)
        for half in range(2):
            for kl in range(16):
                kk = half * 16 + kl
                w3t[kl * 8 : kl * 8 + 8, half * 32 + kk] = c3w[b, kk]
        w3t = w3t.astype(bf)

        b12 = np.stack(
            [
                b1eff[b].reshape(K * C)[0:128],
                b1eff[b].reshape(K * C)[128:256],
                c2b[b].reshape(K * C)[0:128],
                c2b[b].reshape(K * C)[128:256],
            ],
            axis=1,
        ).astype(np.float32)

        b3 = np.tile(b3eff[b], 4)[:, None].astype(np.float32)

        t_m = (target[b] * mf[b][:, None, None]).reshape(K, HW)
        tt_host[b] = np.square(t_m, dtype=np.float64).sum()
        tpk = np.ascontiguousarray(
            t_m.reshape(K, 8, 4, CHUNK).transpose(2, 0, 1, 3).reshape(128, 4096)
        ).astype(bf)

        in_maps.append(
            {
                "f10": f10,
                "w1t": w1t,
                "w2t": w2t,
                "w3t": w3t,
                "b12": b12,
                "b3": b3,
                "tpk": tpk,
            }
        )
    return in_maps, tt_host


def finish(red_list, tt_host):
    per_img = np.empty(B, np.float64)
    for b in range(B):
        r = np.asarray(red_list[b], np.float64)  # [1, 512]
        inter = r[0, 0:256].sum()
        spp = r[0, 256:512].sum()
        stt = tt_host[b]
        per_img[b] = 1.0 - (2.0 * inter + 1.0) / (spp + stt + 1.0)
    return np.float32(per_img.mean())


_NC_CACHE = {}


def kernel(seg_feat, conv_weight, mask, ind, target):
    if "nc" not in _NC_CACHE:
        _NC_CACHE["nc"] = build_nc()
    nc = _NC_CACHE["nc"]
    in_maps, tt_host = prep_inputs(seg_feat, conv_weight, mask, ind, target)
    res = run_bass_kernel_spmd(nc, in_maps, list(range(N_CORES)))
    return finish([res.results[b]["red"] for b in range(B)], tt_host)



# revision 10
# speedup vs baseline: 1.6889x; 1.6889x over previous
"""DiceLoss (CondInst-style dynamic mask head) Trainium2 kernel, v2.

Key insight: mask = randint(0,2) means only ~half the 256 (image, object)
pairs are active; inactive ones contribute exactly zero to the dice sums.
So:
  - Host selects the ~128 active objects, packs them 16-per-tile
    (one partition per object-channel), padded with zero-weight slots.
  - Work is sharded across the 8 cores BY PIXELS (2048 px each), not by
    image: every core runs all active objects on its pixel slice with
    identical weights; per-image partial sums come from an indicator-
    matrix matmul (E[obj, img]) and are combined on host.

Per pixel-chunk the mask head is three matmuls per 16-object tile:
  conv1: contraction = stacked per-image features (8 seg ch x 4 images
         + coords + ones-row carrying the folded bias) at rows 0..34
         (images 0-3) / 64..98 (images 4-7); low/high tiles are paired
         so both stream through disjoint PE row-groups concurrently.
  conv2: dense 128-contraction block-diagonal (16 objects x 8x8).
  conv3: 32-wide col-tiled at 4 PSUM positions (all tiles concurrent).

Evacuation (PSUM->SBUF, the structural bottleneck: only ScalarE+VectorE
can read PSUM) is done in [128,1024] tiles: conv1 evacs span a tile-pair
(pure ReLU, bias pre-folded), conv2 evacs span a chunk-pair of one tile
(per-partition bias+ReLU). Dice products run on GpSimd to keep both
evacuation engines free.
"""

import numpy as np
import ml_dtypes

import concourse.bass as bass
import concourse.mybir as mybir
import concourse.tile as tile
from concourse.bass_utils import run_bass_kernel_spmd

BF16 = mybir.dt.bfloat16
F32 = mybir.dt.float32

B, C, K, H, W = 8, 8, 32, 128, 128
HW = H * W
CW = 169
N_CORES = 8
PX = HW // N_CORES          # 2048 pixels per core
CHUNK = 512
NSUPER = PX // (2 * CHUNK)  # superchunks of 1024 px

_NEG_BIG = 30000.0


# ---------------------------------------------------------------------------
# Workarounds for this walrus build's 1-sem-wait-per-instruction encoding
# limit: split Tile's multi-wait drain and spill excess waits onto NoOps.
# ---------------------------------------------------------------------------
def _drain_and_barrier_split(self, tick_clock, wait_clock):
    from concourse.tile import ScopedClock

    nc = self.nc
    drain_inst = nc.sync.drain()
    wait_clock.add_sem_waits(
        drain_inst.ins, ScopedClock({None: tick_clock.global_clock})
    )
    si = drain_inst.ins.sync_info
    waits = list(si.on_wait) if si is not None else []
    if len(waits) > 1:
        drain_inst.ins.sync_info = None
        handles = list(self.sems.allocated().values())
        by_num = {h.num: h for h in handles}
        by_name = {h.name: h for h in handles}
        for w_ in waits:
            h = by_num.get(w_.id) or by_name.get(w_.ant_name)
            assert h is not None, f"no semaphore handle for {w_}"
            assert w_.wait_mode == "sem-ge-imm", w_.wait_mode
            nc.sync.wait_ge(h, w_.wait_value)
    nc.all_engine_barrier()
    popped = nc._tile_sem_poison_stack.pop()
    assert popped is self._sem_poison
    nc.clear_and_free_semaphores(list(self.sems.allocated().values()))
    nc.all_engine_barrier()


tile.TileContext._drain_and_barrier = _drain_and_barrier_split


def split_excess_waits(nc, register=True):
    for f in nc.m.functions:
        for bb in f.blocks:
            out = []
            changed = False
            for inst in bb.instructions:
                si = inst.sync_info
                waits = list(si.on_wait) if si is not None else []
                if len(waits) > 1:
                    keep, spill = waits[:1], waits[1:]
                    for i, w_ in enumerate(spill):
                        nop = mybir.InstNoOp(
                            name=f"{inst.name}_wspill{i}",
                            engine=inst.engine,
                            sync_info=mybir.SyncInfo(on_wait=[w_], on_update=[]),
                            bass_nofuse=True,
                        )
                        if register:
                            nc.register_instruction(nop, overwrite=True)
                        out.append(nop)
                    inst.sync_info = mybir.SyncInfo(
                        on_wait=keep, on_update=list(si.on_update)
                    )
                    changed = True
                out.append(inst)
            if changed:
                bb.instructions = out


# ---------------------------------------------------------------------------
# Tile plan shared by host packer and device builder
# ---------------------------------------------------------------------------
def make_plan(nt_low, nt_high):
    """Interleave low-group (images 0-3) and high-group (4-7) tiles so
    conv1 pairs stream through disjoint PE row-groups. Returns the tile
    order (list of 'L'/'H') and per-pred-group pair lists."""
    order = []
    for i in range(max(nt_low, nt_high)):
        if i < nt_low:
            order.append("L")
        if i < nt_high:
            order.append("H")
    nt = len(order)
    npg = max(1, (nt + 7) // 8)
    pgs = []
    pair_base = 0
    for pg in range(npg):
        tiles = list(range(8 * pg, min(8 * pg + 8, nt)))
        pairs = []
        for i in range(0, len(tiles), 2):
            ta = tiles[i]
            tb = tiles[i + 1] if i + 1 < len(tiles) else None
            pairs.append((pair_base + len(pairs), ta, tb))
        pgs.append((tiles, pairs))
        pair_base += len(pairs)
    n_pairs = pair_base
    return order, pgs, n_pairs, npg


# ---------------------------------------------------------------------------
# Device kernel
# ---------------------------------------------------------------------------
def build_nc(nt_low, nt_high):
    order, pgs, n_pairs, npg = make_plan(nt_low, nt_high)
    NT = len(order)

    nc = bass.Bass()
    f_d = nc.declare_dram_parameter("f", [128, PX], BF16, False)
    w1_d = nc.declare_dram_parameter("w1", [128, NT * 128], BF16, False)
    w2_d = nc.declare_dram_parameter("w2", [128, NT * 128], BF16, False)
    w3_d = nc.declare_dram_parameter("w3", [128, n_pairs * 64], BF16, False)
    b2_d = nc.declare_dram_parameter("b2", [128, NT], F32, False)
    b3_d = nc.declare_dram_parameter("b3", [128, npg], F32, False)
    e_d = nc.declare_dram_parameter("E", [128, 8 * npg], BF16, False)
    tgt_d = nc.declare_dram_parameter("tgt", [128, npg * PX], BF16, False)
    red_d = nc.declare_dram_parameter("red", [8, 512], F32, True)

    RELU = mybir.ActivationFunctionType.Relu
    SIGM = mybir.ActivationFunctionType.Sigmoid
    ADD = mybir.AluOpType.add
    MAX = mybir.AluOpType.max

    with tile.TileContext(nc) as tc:
        with (
            tc.tile_pool(name="const", bufs=1) as const,
            tc.tile_pool(name="h1p", bufs=4) as h1p,
            tc.tile_pool(name="h2p", bufs=3) as h2p,
            tc.tile_pool(name="predp", bufs=2) as predp,
            tc.tile_pool(name="prodp", bufs=3) as prodp,
            tc.tile_pool(name="ps1p", bufs=1, space="PSUM") as ps1p,
            tc.tile_pool(name="ps2p", bufs=1, space="PSUM") as ps2p,
            tc.tile_pool(name="ps3p", bufs=3, space="PSUM") as ps3p,
            tc.tile_pool(name="psredp", bufs=1, space="PSUM") as psredp,
        ):
            # --- input DMAs (order = need order; two queues) ---
            w1_sb = const.tile([128, NT * 128], BF16)
            nc.gpsimd.dma_start(out=w1_sb[:], in_=w1_d[:])
            f_sb = const.tile([128, PX], BF16)
            nc.gpsimd.dma_start(out=f_sb[:], in_=f_d[:])
            w2_sb = const.tile([128, NT * 128], BF16)
            nc.sync.dma_start(out=w2_sb[:], in_=w2_d[:])
            w3_sb = const.tile([128, n_pairs * 64], BF16)
            nc.sync.dma_start(out=w3_sb[:], in_=w3_d[:])
            b2_sb = const.tile([128, NT], F32)
            nc.sync.dma_start(out=b2_sb[:], in_=b2_d[:])
            b3_sb = const.tile([128, npg], F32)
            nc.sync.dma_start(out=b3_sb[:], in_=b3_d[:])
            e_sb = const.tile([128, 8 * npg], BF16)
            nc.sync.dma_start(out=e_sb[:], in_=e_d[:])
            tgt_sb = const.tile([128, npg * PX], BF16)
            nc.sync.dma_start(out=tgt_sb[:], in_=tgt_d[:])

            red = psredp.tile([8, 512], F32)
            first_red = [True]

            # Bresenham assignment of wide evacs: ACT gets 9/16.
            ev_acc = [0]

            def evac(dst, src, bias_ap):
                ev_acc[0] += 9
                on_act = ev_acc[0] >= 16
                if on_act:
                    ev_acc[0] -= 16
                    if bias_ap is None:
                        nc.scalar.activation(out=dst, in_=src, func=RELU)
                    else:
                        nc.scalar.activation(
                            out=dst, in_=src, func=RELU, bias=bias_ap
                        )
                else:
                    if bias_ap is None:
                        nc.vector.tensor_scalar_max(dst, src, 0.0)
                    else:
                        nc.vector.tensor_scalar(
                            out=dst, in0=src, scalar1=bias_ap, scalar2=0.0,
                            op0=ADD, op1=MAX,
                        )

            for pg_i, (pg_tiles, pg_pairs) in enumerate(pgs):
                pbound = 16 * len(pg_tiles)
                for sc in range(NSUPER):
                    c0 = 2 * sc
                    pred_ps = [
                        ps3p.tile([128, CHUNK], F32, tag="pred_ps",
                                  name="pred_ps")
                        for _ in range(2)
                    ]
                    for pl, (P, ta, tb) in enumerate(pg_pairs):
                        # ---- conv1: tile pair, both chunks
                        h1c = []
                        for ci in range(2):
                            cs = bass.ts(c0 + ci, CHUNK)
                            ps1 = ps1p.tile([128, 1024], F32, tag="ps1",
                                            name="ps1")
                            for half, t in ((0, ta), (1, tb)):
                                if t is None:
                                    continue
                                r0 = 0 if order[t] == "L" else 64
                                nc.tensor.matmul(
                                    ps1[:, bass.ts(half, CHUNK)],
                                    w1_sb[r0 : r0 + 35, bass.ts(t, 128)],
                                    f_sb[r0 : r0 + 35, cs],
                                    start=True, stop=True,
                                    tile_position=(r0, 0),
                                )
                            h1 = h1p.tile([128, 1024], BF16, tag="h1",
                                          name="h1")
                            if tb is None:
                                evac(h1[:, 0:CHUNK], ps1[:, 0:CHUNK], None)
                            else:
                                evac(h1[:], ps1[:], None)
                            h1c.append(h1)
                        # ---- conv2 + conv3 per tile (chunk-paired psum)
                        for half, t in ((0, ta), (1, tb)):
                            if t is None:
                                continue
                            hs = bass.ts(half, CHUNK)
                            ps2 = ps2p.tile([128, 1024], F32, tag="ps2",
                                            name="ps2")
                            for ci in range(2):
                                nc.tensor.matmul(
                                    ps2[:, bass.ts(ci, CHUNK)],
                                    w2_sb[:, bass.ts(t, 128)],
                                    h1c[ci][:, hs],
                                    start=True, stop=True,
                                )
                            h2 = h2p.tile([128, 1024], BF16, tag="h2",
                                          name="h2")
                            evac(h2[:], ps2[:], b2_sb[:, t : t + 1])
                            for ci in range(2):
                                nc.tensor.matmul(
                                    pred_ps[ci][32 * pl : 32 * pl + 32, :],
                                    w3_sb[:, 64 * P + 32 * half :
                                          64 * P + 32 * half + 32],
                                    h2[:, bass.ts(ci, CHUNK)],
                                    start=(half == 0),
                                    stop=(half == 1 or tb is None),
                                    tile_position=(0, 32 * pl),
                                    skip_group_check=True,
                                )
                    # ---- sigmoid + dice products + per-image reduce
                    for ci in range(2):
                        cs_full = pg_i * PX + (c0 + ci) * CHUNK
                        pred = predp.tile([128, CHUNK], BF16, tag="pred",
                                          name="pred")
                        nc.scalar.activation(
                            out=pred[0:pbound, :],
                            in_=pred_ps[ci][0:pbound, :],
                            func=SIGM,
                            bias=b3_sb[0:pbound, pg_i : pg_i + 1],
                        )
                        pt = prodp.tile([128, CHUNK], BF16, tag="pt",
                                        name="pt")
                        nc.gpsimd.tensor_mul(
                            out=pt[0:pbound, :],
                            in0=pred[0:pbound, :],
                            in1=tgt_sb[0:pbound, cs_full : cs_full + CHUNK],
                        )
                        pp = prodp.tile([128, CHUNK], BF16, tag="pp",
                                        name="pp")
                        nc.gpsimd.tensor_mul(
                            out=pp[0:pbound, :],
                            in0=pred[0:pbound, :],
                            in1=pred[0:pbound, :],
                        )
                        for h_ in range(2):
                            nc.tensor.matmul(
                                red[0:8, 0:256],
                                e_sb[0:pbound, 8 * pg_i : 8 * pg_i + 8],
                                pt[0:pbound, bass.ts(h_, 256)],
                                start=first_red[0], stop=False,
                                skip_group_check=True,
                            )
                            first_red[0] = False
                            nc.tensor.matmul(
                                red[0:8, 256:512],
                                e_sb[0:pbound, 8 * pg_i : 8 * pg_i + 8],
                                pp[0:pbound, bass.ts(h_, 256)],
                                start=False, stop=False,
                                skip_group_check=True,
                            )

            red_sb = const.tile([8, 512], F32)
            nc.scalar.copy(out=red_sb[:], in_=red[:])
            nc.gpsimd.dma_start(out=red_d[:], in_=red_sb[:])
    split_excess_waits(nc)
    return nc


# ---------------------------------------------------------------------------
# Host-side input preparation (numpy)
# ---------------------------------------------------------------------------
def prep_inputs(seg_feat, conv_weight, mask, ind, target):
    seg_feat = np.asarray(seg_feat)
    conv_weight = np.asarray(conv_weight)
    mask = np.asarray(mask)
    ind = np.asarray(ind).astype(np.int64)
    target = np.asarray(target)

    cw = conv_weight.reshape(B, CW, HW)
    w = np.take_along_axis(cw, ind[:, None, :], axis=2)  # [B, CW, K]
    w = np.ascontiguousarray(w.transpose(0, 2, 1)).astype(np.float64)

    c1w = w[..., 0:80].reshape(B, K, C, C + 2)
    c1b = w[..., 80:88]
    c2w = w[..., 88:152].reshape(B, K, C, C)
    c2b = w[..., 152:160]
    c3w = w[..., 160:168]
    c3b = w[..., 168]

    x = (ind % W).astype(np.float64) / W
    y = (ind // W).astype(np.float64) / H
    b1eff = c1b - c1w[..., 8] * x[:, :, None] - c1w[..., 9] * y[:, :, None]

    lows = [(b, k) for b in range(4) for k in range(K) if mask[b, k]]
    highs = [(b, k) for b in range(4, 8) for k in range(K) if mask[b, k]]
    nt_low = (len(lows) + 15) // 16
    nt_high = (len(highs) + 15) // 16
    if nt_low + nt_high == 0:
        return None, None, None

    order, pgs, n_pairs, npg = make_plan(nt_low, nt_high)
    NT = len(order)

    # slot table: slots[t][j] = (b, k) or None
    li = hi = 0
    slots = []
    for g in order:
        src = lows if g == "L" else highs
        idx = li if g == "L" else hi
        tile_slots = [src[idx + j] if idx + j < len(src) else None
                      for j in range(16)]
        if g == "L":
            li += 16
        else:
            hi += 16
        slots.append(tile_slots)

    bf = ml_dtypes.bfloat16
    w1t = np.zeros((128, NT * 128), np.float64)
    w2t = np.zeros((128, NT * 128), np.float64)
    w3t = np.zeros((128, n_pairs * 64), np.float64)
    b2t = np.zeros((128, NT), np.float32)
    b3t = np.full((128, npg), -_NEG_BIG, np.float32)
    et = np.zeros((128, 8 * npg), np.float64)
    tgt_all = np.zeros((128, npg, HW), np.float32)

    pair_of_tile = {}
    for pg_tiles, pg_pairs in pgs:
        for P, ta, tb in pg_pairs:
            pair_of_tile[ta] = (P, 0)
            if tb is not None:
                pair_of_tile[tb] = (P, 1)

    for t, tile_slots in enumerate(slots):
        pg_i, tl = divmod(t, 8)
        P, half = pair_of_tile[t]
        for j, slot in enumerate(tile_slots):
            if slot is None:
                continue
            b, k = slot
            rbase = 8 * b if b < 4 else 64 + 8 * (b - 4)
            rc = 32 if b < 4 else 96
            col = 128 * t + 8 * j
            for o in range(C):
                w1t[rbase : rbase + 8, col + o] = c1w[b, k, o, 0:8]
                w1t[rc, col + o] = c1w[b, k, o, 8]
                w1t[rc + 1, col + o] = c1w[b, k, o, 9]
                w1t[rc + 2, col + o] = b1eff[b, k, o]
            w2t[8 * j : 8 * j + 8, col : col + 8] = c2w[b, k].T
            b2t[8 * j : 8 * j + 8, t] = c2b[b, k]
            w3t[8 * j : 8 * j + 8, 64 * P + 32 * half + 16 * half + j] = \
                c3w[b, k]
            prow = 16 * tl + j
            b3t[prow, pg_i] = c3b[b, k]
            et[prow, 8 * pg_i + b] = 1.0
            tgt_all[prow, pg_i, :] = target[b, k].reshape(HW)

    f_all = np.zeros((128, HW), np.float64)
    xg = (np.arange(HW, dtype=np.float64) % W) / W
    yg = (np.arange(HW, dtype=np.float64) // W) / H
    for b in range(B):
        rbase = 8 * b if b < 4 else 64 + 8 * (b - 4)
        f_all[rbase : rbase + 8, :] = seg_feat[b].reshape(C, HW)
    for rc in (32, 96):
        f_all[rc, :] = xg
        f_all[rc + 1, :] = yg
        f_all[rc + 2, :] = 1.0

    w1t = w1t.astype(bf)
    w2t = w2t.astype(bf)
    w3t = w3t.astype(bf)
    et = et.astype(bf)
    f_all = f_all.astype(bf)
    tgt_bf = tgt_all.astype(bf)

    mf = mask.astype(np.float64)[:, :, None, None]
    tt_host = np.square(
        (target.astype(np.float64) * mf).reshape(B, -1)
    ).sum(axis=1)

    in_maps = []
    for core in range(N_CORES):
        sl = slice(core * PX, (core + 1) * PX)
        in_maps.append(
            {
                "f": np.ascontiguousarray(f_all[:, sl]),
                "w1": w1t,
                "w2": w2t,
                "w3": w3t,
                "b2": b2t,
                "b3": b3t,
                "E": et,
                "tgt": np.ascontiguousarray(
                    tgt_bf[:, :, sl].reshape(128, npg * PX)
                ),
            }
        )
    return (nt_low, nt_high), in_maps, tt_host


def finish(red_list, tt_host):
    inter = np.zeros(B, np.float64)
    spp = np.zeros(B, np.float64)
    for r in red_list:
        r = np.asarray(r, np.float64)
        inter += r[:, 0:256].sum(axis=1)
        spp += r[:, 256:512].sum(axis=1)
    per_img = 1.0 - (2.0 * inter + 1.0) / (spp + tt_host + 1.0)
    return np.float32(per_img.mean())


_NC_CACHE = {}


def kernel(seg_feat, conv_weight, mask, ind, target):
    key, in_maps, tt_host = prep_inputs(
        seg_feat, conv_weight, mask, ind, target
    )
    if key is None:  # no active objects: dice = 1 - 1/(tt+1) per image
        tt = np.square(
            np.asarray(target, np.float64)
            * np.asarray(mask, np.float64)[:, :, None, None]
        ).reshape(B, -1).sum(axis=1)
        return np.float32((1.0 - 1.0 / (tt + 1.0)).mean())
    if key not in _NC_CACHE:
        _NC_CACHE[key] = build_nc(*key)
    nc = _NC_CACHE[key]
    res = run_bass_kernel_spmd(nc, in_maps, list(range(N_CORES)))
    return finish(
        [res.results[c]["red"] for c in range(N_CORES)], tt_host
    )


# revision 17
# speedup vs baseline: 2.3165x; 1.3716x over previous
"""DiceLoss (CondInst-style dynamic mask head) Trainium2 kernel, v3.

Key insight: mask = randint(0,2) means only ~half the 256 (image, object)
pairs are active; inactive ones contribute exactly zero to the dice sums.
So:
  - Host selects the ~128 active objects, packs them 16-per-tile
    (one partition per object-channel), padded with zero-weight slots.
  - Work is sharded across the 8 cores BY PIXELS (2048 px each): every
    core runs all active objects on its pixel slice with identical
    weights; per-object partial sums (fused into the GpSimd product
    pass via accum_out) are combined per-image on the host.

Per 512-px chunk the mask head is three matmuls per 16-object tile:
  conv1: contraction = stacked per-image features (8 seg ch x 4 images
         + coords + ones-row carrying the folded bias) at rows 0..34
         (images 0-3) / 64..98 (images 4-7); low/high tiles are paired
         so both stream through disjoint PE row-groups concurrently.
  conv2: dense 128-contraction block-diagonal (16 objects x 8x8).
  conv3: 32-wide col-tiled at 4 PSUM positions (all pairs concurrent).

The emission is software-pipelined over chunk-units u:
conv1(u) interleaves with conv2/conv3(u-1) and sigmoid/products(u-2) so
the PE queue (strict FIFO per engine) never parks behind an instruction
whose PSUM evacuation hasn't finished. PSUM evacuation (only ScalarE +
VectorE can read PSUM) alternates between the two engines; dice
products + free-dim reduction run fused on GpSimd.
"""

import numpy as np
import ml_dtypes

import concourse.bass as bass
import concourse.mybir as mybir
import concourse.tile as tile
from concourse.bass_utils import run_bass_kernel_spmd

BF16 = mybir.dt.bfloat16
F32 = mybir.dt.float32

B, C, K, H, W = 8, 8, 32, 128, 128
HW = H * W
CW = 169
N_CORES = 8
PX = HW // N_CORES          # 2048 pixels per core
CHUNK = 512
NCH = PX // CHUNK           # 4 chunk-units per pred-group

_NEG_BIG = 30000.0


# ---------------------------------------------------------------------------
# Workarounds for this walrus build's 1-sem-wait-per-instruction encoding
# limit: split Tile's multi-wait drain and spill excess waits onto NoOps.
# ---------------------------------------------------------------------------
def _drain_and_barrier_split(self, tick_clock, wait_clock):
    from concourse.tile import ScopedClock

    nc = self.nc
    drain_inst = nc.sync.drain()
    wait_clock.add_sem_waits(
        drain_inst.ins, ScopedClock({None: tick_clock.global_clock})
    )
    si = drain_inst.ins.sync_info
    waits = list(si.on_wait) if si is not None else []
    if len(waits) > 1:
        drain_inst.ins.sync_info = None
        handles = list(self.sems.allocated().values())
        by_num = {h.num: h for h in handles}
        by_name = {h.name: h for h in handles}
        for w_ in waits:
            h = by_num.get(w_.id) or by_name.get(w_.ant_name)
            assert h is not None, f"no semaphore handle for {w_}"
            assert w_.wait_mode == "sem-ge-imm", w_.wait_mode
            nc.sync.wait_ge(h, w_.wait_value)
    nc.all_engine_barrier()
    popped = nc._tile_sem_poison_stack.pop()
    assert popped is self._sem_poison
    nc.clear_and_free_semaphores(list(self.sems.allocated().values()))
    nc.all_engine_barrier()


tile.TileContext._drain_and_barrier = _drain_and_barrier_split


def split_excess_waits(nc, register=True):
    for f in nc.m.functions:
        for bb in f.blocks:
            out = []
            changed = False
            for inst in bb.instructions:
                si = inst.sync_info
                waits = list(si.on_wait) if si is not None else []
                if len(waits) > 1:
                    keep, spill = waits[:1], waits[1:]
                    for i, w_ in enumerate(spill):
                        nop = mybir.InstNoOp(
                            name=f"{inst.name}_wspill{i}",
                            engine=inst.engine,
                            sync_info=mybir.SyncInfo(on_wait=[w_], on_update=[]),
                            bass_nofuse=True,
                        )
                        if register:
                            nc.register_instruction(nop, overwrite=True)
                        out.append(nop)
                    inst.sync_info = mybir.SyncInfo(
                        on_wait=keep, on_update=list(si.on_update)
                    )
                    changed = True
                out.append(inst)
            if changed:
                bb.instructions = out


# ---------------------------------------------------------------------------
# Tile plan shared by host packer and device builder
# ---------------------------------------------------------------------------
def make_plan(nt_low, nt_high):
    """Interleave low-group (images 0-3) and high-group (4-7) tiles so
    conv1 pairs stream through disjoint PE row-groups. Returns the tile
    order (list of 'L'/'H') and per-pred-group pair lists."""
    order = []
    for i in range(max(nt_low, nt_high)):
        if i < nt_low:
            order.append("L")
        if i < nt_high:
            order.append("H")
    nt = len(order)
    npg = max(1, (nt + 7) // 8)
    pgs = []
    pair_base = 0
    for pg in range(npg):
        tiles = list(range(8 * pg, min(8 * pg + 8, nt)))
        pairs = []
        for i in range(0, len(tiles), 2):
            ta = tiles[i]
            tb = tiles[i + 1] if i + 1 < len(tiles) else None
            pairs.append((pair_base + len(pairs), ta, tb))
        pgs.append((tiles, pairs))
        pair_base += len(pairs)
    n_pairs = pair_base
    return order, pgs, n_pairs, npg


# ---------------------------------------------------------------------------
# Device kernel
# ---------------------------------------------------------------------------
def build_nc(nt_low, nt_high):
    order, pgs, n_pairs, npg = make_plan(nt_low, nt_high)
    NT = len(order)
    units = [(pg_i, c) for pg_i in range(npg) for c in range(NCH)]
    NU = len(units)

    nc = bass.Bass()
    f_d = nc.declare_dram_parameter("f", [128, PX], BF16, False)
    w1_d = nc.declare_dram_parameter("w1", [128, NT * 128], BF16, False)
    w2_d = nc.declare_dram_parameter("w2", [128, NT * 128], BF16, False)
    w3_d = nc.declare_dram_parameter("w3", [128, n_pairs * 64], BF16, False)
    b2_d = nc.declare_dram_parameter("b2", [128, NT], F32, False)
    b3_d = nc.declare_dram_parameter("b3", [128, npg], F32, False)
    tgt_d = nc.declare_dram_parameter("tgt", [128, npg * PX], BF16, False)
    part_d = nc.declare_dram_parameter("part", [128, 2 * NU], F32, True)

    RELU = mybir.ActivationFunctionType.Relu
    SIGM = mybir.ActivationFunctionType.Sigmoid
    ADD = mybir.AluOpType.add
    MAX = mybir.AluOpType.max
    MULT = mybir.AluOpType.mult

    with tile.TileContext(nc) as tc:
        with (
            tc.tile_pool(name="const", bufs=1) as const,
            tc.tile_pool(name="h1p", bufs=10) as h1p,
            tc.tile_pool(name="h2p", bufs=6) as h2p,
            tc.tile_pool(name="predp", bufs=2) as predp,
            tc.tile_pool(name="prodp", bufs=3) as prodp,
            tc.tile_pool(name="ps1p", bufs=2, space="PSUM") as ps1p,
            tc.tile_pool(name="ps2p", bufs=2, space="PSUM") as ps2p,
            tc.tile_pool(name="ps3p", bufs=2, space="PSUM") as ps3p,
        ):
            # --- input DMAs spread over four idle engine queues ---
            w1_sb = const.tile([128, NT * 128], BF16)
            nc.gpsimd.dma_start(out=w1_sb[:], in_=w1_d[:])
            f_sb = const.tile([128, PX], BF16)
            nc.gpsimd.dma_start(out=f_sb[:], in_=f_d[:])
            w2_sb = const.tile([128, NT * 128], BF16)
            nc.sync.dma_start(out=w2_sb[:], in_=w2_d[:])
            tgt_sb = const.tile([128, npg * PX], BF16)
            nc.sync.dma_start(out=tgt_sb[:], in_=tgt_d[:])
            w3_sb = const.tile([128, n_pairs * 64], BF16)
            nc.scalar.dma_start(out=w3_sb[:], in_=w3_d[:])
            b3_sb = const.tile([128, npg], F32)
            nc.scalar.dma_start(out=b3_sb[:], in_=b3_d[:])
            b2_sb = const.tile([128, NT], F32)
            nc.scalar.dma_start(out=b2_sb[:], in_=b2_d[:])

            part_sb = const.tile([128, 2 * NU], F32)

            ev_wide = [0]
            ev_nar = [0]

            def evac(dst, src, bias_ap):
                # wides alternate ACT/DVE; narrows go 5/8 to ACT (DVE
                # also runs the fused dice products)
                if bias_ap is None:
                    on_act = ev_wide[0] % 2 == 0
                    ev_wide[0] += 1
                else:
                    ev_nar[0] += 5
                    on_act = ev_nar[0] >= 8
                    if on_act:
                        ev_nar[0] -= 8
                if on_act:
                    if bias_ap is None:
                        nc.scalar.activation(out=dst, in_=src, func=RELU)
                    else:
                        nc.scalar.activation(
                            out=dst, in_=src, func=RELU, bias=bias_ap
                        )
                else:
                    if bias_ap is None:
                        nc.vector.tensor_scalar_max(dst, src, 0.0)
                    else:
                        nc.vector.tensor_scalar(
                            out=dst, in0=src, scalar1=bias_ap, scalar2=0.0,
                            op0=ADD, op1=MAX,
                        )

            # pipeline state per unit
            h1s = {}      # u -> {tile: (h1_tile, colhalf)}
            pred_ps = {}  # u -> psum tile

            def s1_half(u, which):
                """conv1 + evac for pair subset of unit u."""
                pg_i, c = units[u]
                cs = bass.ts(c, CHUNK)
                pairs = pgs[pg_i][1]
                half_n = (len(pairs) + 1) // 2
                sel = pairs[:half_n] if which == 0 else pairs[half_n:]
                hmap = h1s.setdefault(u, {})
                for P, ta, tb in sel:
                    ps1 = ps1p.tile([128, 1024], F32, tag="ps1", name="ps1")
                    for hh, t in ((0, ta), (1, tb)):
                        if t is None:
                            continue
                        r0 = 0 if order[t] == "L" else 64
                        nc.tensor.matmul(
                            ps1[:, bass.ts(hh, CHUNK)],
                            w1_sb[r0 : r0 + 35, bass.ts(t, 128)],
                            f_sb[r0 : r0 + 35, cs],
                            start=True, stop=True,
                            tile_position=(r0, 0),
                        )
                    h1 = h1p.tile([128, 1024], BF16, tag="h1", name="h1")
                    if tb is None:
                        evac(h1[:, 0:CHUNK], ps1[:, 0:CHUNK], None)
                    else:
                        evac(h1[:], ps1[:], None)
                    hmap[ta] = (h1, 0)
                    if tb is not None:
                        hmap[tb] = (h1, 1)

            def s2_conv2_half(u, which):
                """conv2 + evac for tile subset of unit u."""
                pg_i, c = units[u]
                tiles = pgs[pg_i][0]
                half_n = (len(tiles) + 1) // 2
                sel = tiles[:half_n] if which == 0 else tiles[half_n:]
                for t in sel:
                    h1, colhalf = h1s[u][t]
                    ps2 = ps2p.tile([128, CHUNK], F32, tag="ps2", name="ps2")
                    nc.tensor.matmul(
                        ps2[:],
                        w2_sb[:, bass.ts(t, 128)],
                        h1[:, bass.ts(colhalf, CHUNK)],
                        start=True, stop=True,
                    )
                    h2 = h2p.tile([128, CHUNK], BF16, tag="h2", name="h2")
                    evac(h2[:], ps2[:], b2_sb[:, t : t + 1])
                    h1s[u][t] = (h1, colhalf, h2)

            def s2_conv3(u):
                pg_i, c = units[u]
                pairs = pgs[pg_i][1]
                pred = ps3p.tile([128, CHUNK], F32, tag="pred_ps",
                                 name="pred_ps")
                pred_ps[u] = pred
                for pl, (P, ta, tb) in enumerate(pairs):
                    for hh, t in ((0, ta), (1, tb)):
                        if t is None:
                            continue
                        h2 = h1s[u][t][2]
                        nc.tensor.matmul(
                            pred[32 * pl : 32 * pl + 32, :],
                            w3_sb[:, 64 * P + 32 * hh : 64 * P + 32 * hh + 32],
                            h2[:],
                            start=(hh == 0),
                            stop=(hh == 1 or tb is None),
                            tile_position=(0, 32 * pl),
                            skip_group_check=True,
                        )

            def s3(u):
                pg_i, c = units[u]
                pbound = 16 * len(pgs[pg_i][0])
                pred_sb = predp.tile([128, CHUNK], BF16, tag="pred",
                                     name="pred")
                nc.scalar.activation(
                    out=pred_sb[0:pbound, :],
                    in_=pred_ps[u][0:pbound, :],
                    func=SIGM,
                    bias=b3_sb[0:pbound, pg_i : pg_i + 1],
                )
                del pred_ps[u]
                tcol = pg_i * PX + c * CHUNK
                sc1 = prodp.tile([128, CHUNK], BF16, tag="sc1", name="sc1")
                nc.gpsimd.tensor_mul(
                    out=sc1[0:pbound, :],
                    in0=pred_sb[0:pbound, :],
                    in1=tgt_sb[0:pbound, tcol : tcol + CHUNK],
                )
                nc.vector.tensor_reduce(
                    out=part_sb[0:pbound, 2 * u : 2 * u + 1],
                    in_=sc1[0:pbound, :],
                    axis=mybir.AxisListType.X,
                    op=ADD,
                )
                sc2 = prodp.tile([128, CHUNK], BF16, tag="sc2", name="sc2")
                nc.gpsimd.tensor_mul(
                    out=sc2[0:pbound, :],
                    in0=pred_sb[0:pbound, :],
                    in1=pred_sb[0:pbound, :],
                )
                nc.vector.tensor_reduce(
                    out=part_sb[0:pbound, 2 * u + 1 : 2 * u + 2],
                    in_=sc2[0:pbound, :],
                    axis=mybir.AxisListType.X,
                    op=ADD,
                )
                del h1s[u]

            # --- software-pipelined emission ---
            for step in range(NU + 2):
                if step < NU:
                    s1_half(step, 0)
                if 0 <= step - 1 < NU:
                    s2_conv2_half(step - 1, 0)
                if step < NU:
                    s1_half(step, 1)
                if 0 <= step - 1 < NU:
                    s2_conv2_half(step - 1, 1)
                    s2_conv3(step - 1)
                if 0 <= step - 2 < NU:
                    s3(step - 2)

            nc.gpsimd.dma_start(out=part_d[:], in_=part_sb[:])
    split_excess_waits(nc)
    return nc


# ---------------------------------------------------------------------------
# Host-side input preparation (numpy)
# ---------------------------------------------------------------------------
def prep_inputs(seg_feat, conv_weight, mask, ind, target):
    seg_feat = np.asarray(seg_feat)
    conv_weight = np.asarray(conv_weight)
    mask = np.asarray(mask)
    ind = np.asarray(ind).astype(np.int64)
    target = np.asarray(target)

    cw = conv_weight.reshape(B, CW, HW)
    w = np.take_along_axis(cw, ind[:, None, :], axis=2)  # [B, CW, K]
    w = np.ascontiguousarray(w.transpose(0, 2, 1)).astype(np.float64)

    c1w = w[..., 0:80].reshape(B, K, C, C + 2)
    c1b = w[..., 80:88]
    c2w = w[..., 88:152].reshape(B, K, C, C)
    c2b = w[..., 152:160]
    c3w = w[..., 160:168]
    c3b = w[..., 168]

    x = (ind % W).astype(np.float64) / W
    y = (ind // W).astype(np.float64) / H
    b1eff = c1b - c1w[..., 8] * x[:, :, None] - c1w[..., 9] * y[:, :, None]

    lows = [(b, k) for b in range(4) for k in range(K) if mask[b, k]]
    highs = [(b, k) for b in range(4, 8) for k in range(K) if mask[b, k]]
    nt_low = (len(lows) + 15) // 16
    nt_high = (len(highs) + 15) // 16
    if nt_low + nt_high == 0:
        return None, None, None, None

    order, pgs, n_pairs, npg = make_plan(nt_low, nt_high)
    NT = len(order)

    li = hi = 0
    slots = []
    for g in order:
        src = lows if g == "L" else highs
        idx = li if g == "L" else hi
        tile_slots = [src[idx + j] if idx + j < len(src) else None
                      for j in range(16)]
        if g == "L":
            li += 16
        else:
            hi += 16
        slots.append(tile_slots)

    bf = ml_dtypes.bfloat16
    w1t = np.zeros((128, NT * 128), np.float64)
    w2t = np.zeros((128, NT * 128), np.float64)
    w3t = np.zeros((128, n_pairs * 64), np.float64)
    b2t = np.zeros((128, NT), np.float32)
    b3t = np.full((128, npg), -_NEG_BIG, np.float32)
    tgt_all = np.zeros((128, npg, HW), np.float32)
    img_of_row = np.full((npg, 128), -1, np.int32)

    pair_of_tile = {}
    for pg_tiles, pg_pairs in pgs:
        for P, ta, tb in pg_pairs:
            pair_of_tile[ta] = (P, 0)
            if tb is not None:
                pair_of_tile[tb] = (P, 1)

    for t, tile_slots in enumerate(slots):
        pg_i, tl = divmod(t, 8)
        P, half = pair_of_tile[t]
        for j, slot in enumerate(tile_slots):
            if slot is None:
                continue
            b, k = slot
            rbase = 8 * b if b < 4 else 64 + 8 * (b - 4)
            rc = 32 if b < 4 else 96
            col = 128 * t + 8 * j
            for o in range(C):
                w1t[rbase : rbase + 8, col + o] = c1w[b, k, o, 0:8]
                w1t[rc, col + o] = c1w[b, k, o, 8]
                w1t[rc + 1, col + o] = c1w[b, k, o, 9]
                w1t[rc + 2, col + o] = b1eff[b, k, o]
            w2t[8 * j : 8 * j + 8, col : col + 8] = c2w[b, k].T
            b2t[8 * j : 8 * j + 8, t] = c2b[b, k]
            w3t[8 * j : 8 * j + 8, 64 * P + 32 * half + 16 * half + j] = \
                c3w[b, k]
            prow = 16 * tl + j
            b3t[prow, pg_i] = c3b[b, k]
            img_of_row[pg_i, prow] = b
            tgt_all[prow, pg_i, :] = target[b, k].reshape(HW)

    f_all = np.zeros((128, HW), np.float64)
    xg = (np.arange(HW, dtype=np.float64) % W) / W
    yg = (np.arange(HW, dtype=np.float64) // W) / H
    for b in range(B):
        rbase = 8 * b if b < 4 else 64 + 8 * (b - 4)
        f_all[rbase : rbase + 8, :] = seg_feat[b].reshape(C, HW)
    for rc in (32, 96):
        f_all[rc, :] = xg
        f_all[rc + 1, :] = yg
        f_all[rc + 2, :] = 1.0

    w1t = w1t.astype(bf)
    w2t = w2t.astype(bf)
    w3t = w3t.astype(bf)
    f_all = f_all.astype(bf)
    tgt_bf = tgt_all.astype(bf)

    mf = mask.astype(np.float64)[:, :, None, None]
    tt_host = np.square(
        (target.astype(np.float64) * mf).reshape(B, -1)
    ).sum(axis=1)

    in_maps = []
    for core in range(N_CORES):
        sl = slice(core * PX, (core + 1) * PX)
        in_maps.append(
            {
                "f": np.ascontiguousarray(f_all[:, sl]),
                "w1": w1t,
                "w2": w2t,
                "w3": w3t,
                "b2": b2t,
                "b3": b3t,
                "tgt": np.ascontiguousarray(
                    tgt_bf[:, :, sl].reshape(128, npg * PX)
                ),
            }
        )
    return (nt_low, nt_high), in_maps, tt_host, img_of_row


def finish(part_list, tt_host, img_of_row):
    npg = img_of_row.shape[0]
    inter = np.zeros(B, np.float64)
    spp = np.zeros(B, np.float64)
    for part in part_list:
        part = np.asarray(part, np.float64)  # [128, 2*NU]
        for u in range(part.shape[1] // 2):
            pg_i = u // NCH
            for b in range(B):
                rows = img_of_row[pg_i] == b
                inter[b] += part[rows, 2 * u].sum()
                spp[b] += part[rows, 2 * u + 1].sum()
    per_img = 1.0 - (2.0 * inter + 1.0) / (spp + tt_host + 1.0)
    return np.float32(per_img.mean())


_NC_CACHE = {}


def kernel(seg_feat, conv_weight, mask, ind, target):
    key, in_maps, tt_host, img_of_row = prep_inputs(
        seg_feat, conv_weight, mask, ind, target
    )
    if key is None:  # no active objects: dice = 1 - 1/(tt+1) per image
        tt = np.square(
            np.asarray(target, np.float64)
            * np.asarray(mask, np.float64)[:, :, None, None]
        ).reshape(B, -1).sum(axis=1)
        return np.float32((1.0 - 1.0 / (tt + 1.0)).mean())
    if key not in _NC_CACHE:
        _NC_CACHE[key] = build_nc(*key)
    nc = _NC_CACHE[key]
    res = run_bass_kernel_spmd(nc, in_maps, list(range(N_CORES)))
    return finish(
        [res.results[c]["part"] for c in range(N_CORES)],
        tt_host, img_of_row,
    )
